# revision 29
# baseline (speedup 1.0000x reference)
"""AttentionRNN (BiDAF-style QA reader) Trainium2 kernel, v2.

Per core (pure data-parallel over batch, 4 of 32 rows per core):
  1. Host gathers embeddings (padded to 384 rows; column 300 is a pad-token
     indicator that the z-gate weight row turns into a +1e4 logit, freezing
     h across padding exactly, replacing any explicit mask tensor).
  2. xp projections for the 4 GRU directions (bf16 PE) written in
     (gate, step, chunk, batch) layout with warmup pad chunks per region so
     every per-round slice is contiguous; pad-chunk x is memset (BIGM in z)
     so warmup-frozen chains freeze via the same z-pin mechanism.
  3. GRU scan, chunked-parallel: payload chunks of 16 (passage) / 8
     (question) steps, W=10 warmup steps re-run from h=0 (warmup error is
     below the bf16 noise floor).  Two software streams (fwd dirs / bwd
     dirs) run half-a-round offset.  Per round, x slices enter PSUM via
     identity-matmul accumulation; whh @ h' is decomposed by linearity into
     whh@u + whh@v (u = n + z*h, v = -z*n) so the PE input is ready one
     elementwise op after tanh; h' materializes off the critical path and
     payload h' writes the encoder slots directly.
  4. Attention: transposed-logit formulation (softmax row constants w1.p
     and attn_b cancel), per-batch exp on [64,P] tiles with normalization
     deferred through the head matmuls (block-sparse ones lhs accumulates
     per-(head,b) column sums into rows 0:8; se = seA + seB * 1/sums),
     heads via block-sparse lhs weights into rows 0:8, log-softmax tail.
     All PE ops keep partition base 0/32/64 (base 96 and cross-base lhs/rhs
     crash the device).
"""

import contextlib

import numpy as np
import ml_dtypes

import concourse.bass as bass
import concourse.mybir as mybir
from concourse.masks import make_identity
from concourse.tile import TileContext
from concourse.bass_utils import run_bass_kernel_spmd

F32 = mybir.dt.float32
BF16 = mybir.dt.bfloat16
U8 = mybir.dt.uint8
AX = mybir.AxisListType.X
ALU = mybir.AluOpType
AF = mybir.ActivationFunctionType

B, P, Q, E, H, VOCAB = 32, 512, 64, 300, 256, 50000
HH = 128
EPAD = 384
E2 = 64  # third-kc-chunk rows, 45 real (dims 256..299 + indicator) padded
E2R = 45  # real rows in the third chunk
NC = 8
BC = B // NC
NEG = -1e7
BIGM = 1.0e4

import os
SP_ = 16
SQ_ = int(os.environ.get("KSQ", "8"))  # q payload chunk size
NCP, NCQ = P // SP_, Q // SQ_   # 32, 8 chunks
W = int(os.environ.get("KW", "7"))  # warmup rounds (contraction ~0.65/step)
PADP = (W + SP_ - 1) // SP_     # warmup pad chunks
PADQ = (W + SQ_ - 1) // SQ_
CPP, CPQ = NCP + PADP, NCQ + PADQ
RND = W + SP_                # total rounds
NTP, NTQ = BC * P, BC * Q    # 2048, 256

# x tile free-dim strides (elements), layout [128, gate(3), s, cpad, b]
XPG, XPS = SP_ * CPP * BC, CPP * BC     # 2176, 136
XQG, XQS = SQ_ * CPQ * BC, CPQ * BC     # 320, 40
# enc layout [128, s(16), c(40), b(4)]: p chunks 0:32, q chunks 32:40
ENCC = (NCP + NCQ) * BC                 # 160 cols per s-row

_CACHE = {}

V_ACCUM = os.environ.get("KV_ACCUM", "1") == "1"   # exp accum_out in lsm
V_INPLACE = os.environ.get("KV_INPLACE", "0") == "1"  # t1 in-place psum + PE xn
V_SIG3D = os.environ.get("KV_SIG3D", "0") == "1"   # single 3D-AP sigmoid
_PH = int(os.environ.get("KPH", "3"))  # 1=projections 2=+scan 3=full
KPRE_N = int(os.environ.get("KPRE", "1"))  # 0=pre-tc barrier, 1=in-tc gated, 2=barrier+waits
KPRE = KPRE_N >= 1  # waits emitted for modes 1,2


def _build_nc():
    nc = bass.Bass()

    epTp_d = nc.declare_dram_parameter("epTp_d", [128, 2 * NTP], BF16,
                                       isOutput=False)
    epTp2_d = nc.declare_dram_parameter("epTp2_d", [E2, NTP], BF16,
                                        isOutput=False)
    epTq_d = nc.declare_dram_parameter("epTq_d", [128, 2 * NTQ], BF16,
                                       isOutput=False)
    epTq2_d = nc.declare_dram_parameter("epTq2_d", [E2, NTQ], BF16,
                                        isOutput=False)
    wihT = nc.declare_dram_parameter("wihT", [128, 4 * 2 * 3 * HH], BF16,
                                     isOutput=False)
    wih2T = nc.declare_dram_parameter("wih2T", [E2, 4 * 3 * HH], BF16,
                                      isOutput=False)
    whhT = nc.declare_dram_parameter("whhT", [128, 4 * 3 * HH], BF16,
                                     isOutput=False)
    brzn = nc.declare_dram_parameter("brzn", [128, 12], F32, isOutput=False)
    bhnr = nc.declare_dram_parameter("bhnr", [1, 576], BF16, isOutput=False)
    qm0d = nc.declare_dram_parameter("qm0", [1, NTQ], F32, isOutput=False)
    seww = nc.declare_dram_parameter("sew", [HH, 14], BF16, isOutput=False)
    sew24 = nc.declare_dram_parameter("sew24", [HH, 192], BF16,
                                      isOutput=False)
    ones8 = nc.declare_dram_parameter("ones8", [128, 32], BF16,
                                      isOutput=False)
    outw = nc.declare_dram_parameter("outw", [HH, 4], F32, isOutput=False)
    m8 = nc.declare_dram_parameter("m8", [2 * BC, P], U8, isOutput=False)
    out = nc.declare_dram_parameter("out", [4 * BC, P], F32, isOutput=True)

    es = contextlib.ExitStack()

    def sb(name, shape, dtype):
        return es.enter_context(nc.sbuf_tensor(name, shape, dtype))

    # raw sbuf: written only in the pre-Tile preamble
    epTp = sb("epTp", [128, 2, NTP], BF16)
    epTp2 = sb("epTp2", [E2, NTP], BF16)
    epTq = sb("epTq", [128, 2, NTQ], BF16)
    epTq2 = sb("epTq2", [E2, NTQ], BF16)
    wih_sb = sb("wih_sb", [128, 4 * 2 * 3 * HH], BF16)
    wih2_sb = sb("wih2_sb", [E2, 4 * 3 * HH], BF16)
    whh_sb = sb("whh_sb", [128, 4 * 3 * HH], BF16)
    brzn_sb = sb("brzn_sb", [128, 12], F32)
    bhnr_sb = sb("bhnr_sb", [1, 576], BF16)
    qm0_sb = sb("qm0_sb", [1, NTQ], F32)
    sew_sb = sb("sew_sb", [128, 14], BF16)
    sew24_sb = sb("sew24_sb", [128, 192], BF16)
    outw_sb = sb("outw_sb", [128, 4], F32)
    m8_sb = sb("m8_sb", [2 * BC, P], U8)
    neg_sb = sb("neg_sb", [2 * BC, P], F32)
    ones_sb = sb("ones_sb", [128, 512], BF16)
    ones8_sb = sb("ones8_sb", [128, 32], BF16)
    ident_sb = sb("ident_sb", [128, 128], BF16)

    crit_sem = es.enter_context(nc.semaphore("crit_sem"))
    eptp_sem = es.enter_context(nc.semaphore("eptp_sem"))
    aux_sem = es.enter_context(nc.semaphore("aux_sem"))
    init_sem = es.enter_context(nc.semaphore("init_sem"))

    def dmalist():
        # aux positions: brzn=16 bhnr=32 qm0=48 m8=64 sew=80 outw=96
        # whh=112 sew24=128 ones8=144
        return ((brzn_sb[:, :], brzn[:, :], aux_sem),
                (bhnr_sb[:, :], bhnr[:, :], aux_sem),
                (wih_sb[:, :], wihT[:, :], crit_sem),
                (epTq[:, 0, :], epTq_d[:, 0:NTQ], crit_sem),
                (epTq[:, 1, :], epTq_d[:, NTQ:2 * NTQ], crit_sem),
                (epTq2[:, :], epTq2_d[:, :], crit_sem),
                (wih2_sb[:, :], wih2T[:, :], crit_sem),
                (epTp[:, 0, :], epTp_d[:, 0:NTP], eptp_sem),
                (epTp[:, 1, :], epTp_d[:, NTP:2 * NTP], eptp_sem),
                (epTp2[:, :], epTp2_d[:, :], eptp_sem),
                (qm0_sb[:, :], qm0d[:, :], aux_sem),
                (m8_sb[:, :], m8[:, :], aux_sem),
                (sew_sb[0:HH, :], seww[:, :], aux_sem),
                (outw_sb[0:HH, :], outw[:, :], aux_sem),
                (whh_sb[:, :], whhT[:, :], aux_sem),
                (sew24_sb[0:HH, :], sew24[:, :], aux_sem),
                (ones8_sb[:, :], ones8[:, :], aux_sem))

    def make_ident(eng):
        eng.memset(ident_sb[:, :], 0.0)
        eng.affine_select(
            out=ident_sb[:, :], in_=ident_sb[:, :],
            compare_op=ALU.not_equal, fill=1.0, base=0,
            pattern=[[-1, 128]], channel_multiplier=1)

    if KPRE_N in (0, 2):
        # baseline-style pre-tc preamble: all loads on gpsimd SWDGE, one
        # global barrier; no in-tc gating.
        for _dst, _src, _sem in dmalist():
            nc.gpsimd.dma_start(out=_dst, in_=_src).then_inc(_sem, 16)
        make_ident(nc.gpsimd)
        nc.vector.memset(neg_sb[:, :], NEG)
        nc.vector.memset(ones_sb[:, :], 1.0)
        nc.vector.wait_ge(aux_sem, 144)
        nc.vector.wait_ge(crit_sem, 80)
        nc.vector.wait_ge(eptp_sem, 48)
        nc.vector.drain()
        cmb_sem = es.enter_context(nc.semaphore("cmb_sem"))
        nc.vector.sem_inc(cmb_sem, 1)
        for eng in (nc.scalar, nc.tensor, nc.gpsimd, nc.sync):
            eng.wait_ge(cmb_sem, 1)

    with TileContext(nc) as tc:
        with tc.tile_pool(name="ps", bufs=2, space="PSUM") as ps, \
             tc.tile_pool(name="sb", bufs=2) as sbp, \
             tc.tile_pool(name="pst", bufs=1) as pst:

            if KPRE_N in (1, 3):
                # ---- raw preamble (in-block so the tile scheduler models
                # the semaphores).  All loads ride the sync/HWDGE queue in
                # strict bus-priority order: tiny biases first (they unblock
                # the projection moves), then wih+epTq (q projections), the
                # three epTp chunks (p pjobs, gated per chunk), and finally
                # the small/late loads whose consumers (scan round 1 /
                # attention) run tens of microseconds later.  SWDGE (gpsimd)
                # DMA is unusable inside a tile block: tile emits
                # InstIncSwdgeSem doorbells this walrus build cannot encode.
                deng = nc.scalar if os.environ.get("KDENG", "sync") == "scalar" else nc.sync
                for _dst, _src, _sem in dmalist():
                    deng.dma_start(out=_dst, in_=_src).then_inc(_sem, 16)
                # identity on gpsimd (free of DMA work now)
                make_ident(nc.gpsimd)
                nc.gpsimd.sem_inc(init_sem, 1)
                nc.vector.memset(neg_sb[:, :], NEG)
                nc.vector.memset(ones_sb[:, :], 1.0)
                nc.vector.sem_inc(init_sem, 1)
                if KPRE_N == 3:  # coarse: every engine waits for everything
                    for eng in (nc.tensor, nc.scalar, nc.vector, nc.gpsimd):
                        eng.wait_ge(crit_sem, 80)
                        eng.wait_ge(eptp_sem, 48)
                        eng.wait_ge(aux_sem, 144)
                        eng.wait_ge(init_sem, 2)

            def pt(name, shape, dtype):
                return pst.tile(shape, dtype, name=name, tag=name)

            ident = ident_sb

            xPA = pt("xPA", [128, 3, SP_, CPP, BC], BF16)
            xPB = pt("xPB", [128, 3, SP_, CPP, BC], BF16)
            xQA = pt("xQA", [128, 3, SQ_, CPQ, BC], BF16)
            xQB = pt("xQB", [128, 3, SQ_, CPQ, BC], BF16)
            encA = pt("encA", [128, SP_, ENCC // BC, BC], BF16)
            encB = pt("encB", [128, SP_, ENCC // BC, BC], BF16)
            hA = pt("hA", [128, ENCC], BF16)
            hB = pt("hB", [128, ENCC], BF16)

            pencFB = pt("pencFB", [128, 2 * NTP], BF16)
            qencFB = pt("qencFB", [128, 2 * NTQ], BF16)
            qenc3 = pt("qenc3", [128, 2 * NTQ], BF16)
            qencT = pt("qencT", [64, 8 * HH], BF16)
            qwm = pt("qwm", [1, NTQ], BF16)
            rs8_sb = pt("rs8_sb", [2 * BC, P], F32)
            attwFB = pt("attwFB", [128, 2 * NTP], BF16)
            pawFB = pt("pawFB", [128, 2 * NTP], BF16)
            se8 = pt("se8", [2 * BC, P], F32)
            lsm_sb = pt("lsm_sb", [2 * BC, P], F32)
            lse_sb = pt("lse_sb", [2 * BC, P], F32)
            red_sb = pt("red_sb", [2 * BC, 8], F32)

            nc.vector.memset(hA[:, :], 0)
            nc.vector.memset(hB[:, :], 0)
            # x pad chunks: r/n gates -> 0 ; z gate -> BIGM (freeze)
            for xt, np_, c0 in ((xPA, PADP, 0), (xPB, PADP, NCP),
                                (xQA, PADQ, 0), (xQB, PADQ, NCQ)):
                spad = xt[:, :, :, c0:c0 + np_, :]
                nc.vector.memset(spad[:, 0, :, :, :], 0)
                nc.vector.memset(spad[:, 2, :, :, :], 0)
                nc.vector.memset(spad[:, 1, :, :, :], BIGM)
            if KPRE_N == 1:
                nc.scalar.wait_ge(aux_sem, 16)
                nc.vector.wait_ge(aux_sem, 16)

            # ---- projections ----
            # p dirs: per (dir, batch, gate): psum [128,512] = sum_kc wih.T@ep
            # (GPSIMD cannot access PSUM, so moves rotate Act/DVE only)
            mveng = [nc.scalar, nc.vector, nc.gpsimd]
            mvi = 0

            def move(dst, src, bias_ap):
                nonlocal mvi
                eng = mveng[mvi % 2]
                mvi += 1
                if eng is nc.scalar:
                    eng.activation(dst, src, AF.Identity, bias=bias_ap)
                else:
                    eng.tensor_scalar_add(dst, src, bias_ap)

            # q dirs projected fully upfront (all q steps feed rounds 0-7),
            # kc-major so matmuls start as soon as each input chunk lands
            for di, (xt, c0) in ((2, (xQA, PADQ)), (3, (xQB, 0))):
                pps = [ps.tile([128, 512], F32, name="pj", tag="pj",
                               bufs=3) for _ in range(3)]
                for kc in range(3):
                    if KPRE_N == 1 and di == 2:
                        nc.tensor.wait_ge(crit_sem, (32, 48, 80)[kc])
                    for g in range(3):
                        if kc < 2:
                            wcol = ((di * 2 + kc) * 3 + g) * HH
                            lhs = wih_sb[:, wcol:wcol + HH]
                            rhs = epTq[:, kc, :]
                        else:
                            wcol = (di * 3 + g) * HH
                            lhs = wih2_sb[:, wcol:wcol + HH]
                            rhs = epTq2[:, :]
                        nc.tensor.matmul(pps[g][:, 0:NTQ], lhs, rhs,
                                         start=(kc == 0), stop=(kc == 2))
                for g in range(3):
                    dst = xt[:, g, :, c0:c0 + NCQ, :] \
                        .rearrange("p s c b -> p b c s")
                    move(dst, pps[g][:, 0:NTQ],
                         brzn_sb[:, di * 3 + g:di * 3 + g + 1])

            # p dirs as per-2-step jobs in consumption (pincer) order:
            # stream A consumes s=8..15,0..7; B consumes s=7..0,15..8
            def _pjob(di, xt, c0, sb0, gate=False):
                def f():
                    rhs01 = epTp.rearrange("p k (b c s) -> p k b c s",
                                           b=BC, s=SP_)
                    rhs2 = epTp2.rearrange("p (b c s) -> p b c s",
                                           b=BC, s=SP_)
                    pps = [ps.tile([128, 512], F32, name="pj", tag="pj",
                                   bufs=3) for _ in range(3)]
                    for kc in range(3):
                        if KPRE_N == 1 and gate:
                            nc.tensor.wait_ge(eptp_sem, (kc + 1) * 16)
                        for g in range(3):
                            if kc < 2:
                                wcol = ((di * 2 + kc) * 3 + g) * HH
                                lhs = wih_sb[:, wcol:wcol + HH]
                                rhs = rhs01[:, kc, :, :, sb0:sb0 + 2]
                            else:
                                wcol = (di * 3 + g) * HH
                                lhs = wih2_sb[:, wcol:wcol + HH]
                                rhs = rhs2[:, :, :, sb0:sb0 + 2]
                            nc.tensor.matmul(
                                pps[g][:, 0:2 * NCP * BC], lhs, rhs,
                                start=(kc == 0), stop=(kc == 2))
                    for g in range(3):
                        dst = xt[:, g, sb0:sb0 + 2, c0:c0 + NCP, :] \
                            .rearrange("p s c b -> p b c s")
                        move(dst, pps[g][:, 0:2 * NCP * BC],
                             brzn_sb[:, di * 3 + g:di * 3 + g + 1])
                return f

            _pjobs = []
            a0 = ((SP_ - W) % SP_) // 2 * 2  # A's first block (even-aligned)
            b0 = ((W - 1) // 2) * 2          # B's first block
            for j in range(8):
                sA = (a0 + 2 * j) % SP_      # A consumes s ascending
                sB = (b0 - 2 * j) % SP_      # B consumes 15-s descending
                _pjobs.append(_pjob(0, xPA, PADP, sA, gate=(j == 0)))
                _pjobs.append(_pjob(1, xPB, 0, sB))
            for _ in range(2):
                _pjobs.pop(0)()
            if KPRE_N == 1:
                # bhnr(aux 32) + ident/ones round 0; whh arrives by round 1
                nc.tensor.wait_ge(aux_sem, 32)
                nc.tensor.wait_ge(init_sem, 2)

            if _PH < 2:
                nc.gpsimd.dma_start(out[0:16, :], xPA[0:16, 0, 0:4, PADP:PADP + 32, :])
            # ---- GRU scan ----
            # psum bank layout per stream/round: r[0:160) z[160:320) n[320:480)
            OFR, OFZ, OFN = 0, 160, 320
            SPRM = {"A": (xPA, xQA, encA, hA, 0, 2),
                    "B": (xPB, xQB, encB, hB, 1, 3)}
            _stash = {}

            def geom(st, k):
                e = k - W
                s, coff = e % SP_, e // SP_
                sq, coffq = e % SQ_, e // SQ_
                qact = e < SQ_
                wd = ENCC if qact else NCP * BC
                if st == "A":
                    return e, qact, wd, PADP + coff, PADQ + coffq, s, sq, s
                return (e, qact, wd, -coff, -coffq,
                        SP_ - 1 - s, SQ_ - 1 - sq, SP_ - 1 - s)

            def hsrc_of(st, k):
                xp, xq, enc, hcu, dp, dq = SPRM[st]
                e, qact, wd = geom(st, k)[:3]
                if e - 1 < 0:
                    return hcu[:, 0:wd]
                rprev = (e - 1) if st == "A" else (SP_ - e)
                pcc = ENCC // BC if e - 1 < SQ_ else NCP
                return enc[:, rprev, 0:pcc, :].rearrange(
                    "p c b -> p (c b)")[:, 0:wd]

            def first_half(st, k):
                xp, xq, enc, hcu, dp, dq = SPRM[st]
                e, qact, wd, cp0, cq0, sx, sxq, row = geom(st, k)
                pm = ps.tile([128, 512], F32, name="prz" + st, tag=st)
                mm = nc.tensor.matmul
                first = [True]

                def gmm(dst, lhs, rhs, stop=False):
                    mm(dst, lhs, rhs, start=first[0], stop=stop)
                    first[0] = False

                for g, off in ((0, OFR), (1, OFZ)):
                    gmm(pm[:, off:off + NCP * BC], ident[:, :],
                        xp[:, g, sx, cp0:cp0 + NCP, :])
                    if qact:
                        gmm(pm[:, off + NCP * BC:off + wd], ident[:, :],
                            xq[:, g, sxq, cq0:cq0 + NCQ, :])
                # whh @ h'(k-1) via linearity: h' = u + v (u = n + z*h,
                # v = -z*n); whh@u issues as soon as u is ready, whh@v last.
                if k > 0:
                    for src_t, wtb in zip(_uv[st], (whh_sb, whh_sb)):
                        hp = src_t[:, 0:NCP * BC]
                        hq = src_t[:, NCP * BC:wd] if qact else None
                        for g, off in ((0, OFR), (1, OFZ), (2, OFN)):
                            wc = (dp * 3 + g) * HH
                            gmm(pm[:, off:off + NCP * BC],
                                wtb[:, wc:wc + HH], hp)
                            if qact:
                                wcq = (dq * 3 + g) * HH
                                gmm(pm[:, off + NCP * BC:off + wd],
                                    wtb[:, wcq:wcq + HH], hq)
                gmm(pm[:, OFN:OFN + NCP * BC],
                    bhnr_sb[0:1, dp * HH:(dp + 1) * HH],
                    ones_sb[0:1, 0:NCP * BC], stop=not qact)
                if qact:
                    gmm(pm[:, OFN + NCP * BC:OFN + wd],
                        bhnr_sb[0:1, dq * HH:(dq + 1) * HH],
                        ones_sb[0:1, 0:NCQ * BC], stop=True)

                # sigmoid r|z in one op
                rz = sbp.tile([128, 2, 160], BF16, name="rz" + st,
                              tag="rz" + st)
                nc.scalar.activation(
                    rz[:, :, 0:wd],
                    pm[:, 0:2 * 160].rearrange(
                        "p (g x) -> p g x", x=160)[:, :, 0:wd],
                    AF.Sigmoid)
                # off-chain: zh = z*h'(k-1)
                zh = sbp.tile([128, 160], BF16, name="zh" + st,
                              tag="zh" + st)
                if k > 0:
                    nc.gpsimd.tensor_mul(zh[:, 0:wd], rz[:, 1, 0:wd],
                                         _hh[st][:, 0:wd])
                # t1 = pn * r in place; += xn via PE identity accumulate
                nc.vector.tensor_mul(pm[:, OFN:OFN + wd],
                                     pm[:, OFN:OFN + wd], rz[:, 0, 0:wd])
                mm(pm[:, OFN:OFN + NCP * BC], ident[:, :],
                   xp[:, 2, sx, cp0:cp0 + NCP, :],
                   start=False, stop=False, skip_group_check=True)
                if qact:
                    mm(pm[:, OFN + NCP * BC:OFN + wd], ident[:, :],
                       xq[:, 2, sxq, cq0:cq0 + NCQ, :],
                       start=False, stop=False, skip_group_check=True)
                _stash[st] = (pm, rz, zh)

            def second_half(st, k):
                xp, xq, enc, hcu, dp, dq = SPRM[st]
                e, qact, wd, cp0, cq0, sx, sxq, row = geom(st, k)
                pm, rz, zh = _stash[st]
                nt = sbp.tile([128, 160], BF16, name="nt" + st,
                              tag="nt" + st)
                nc.scalar.activation(nt[:, 0:wd], pm[:, OFN:OFN + wd],
                                     AF.Tanh)
                # v = -z*n (fused negate: whh@v replaces the old whhN@(z*n))
                # then u = n + zh, back-to-back on DVE
                vt = sbp.tile([128, 160], BF16, name="vt" + st,
                              tag="vt" + st)
                nc.vector.scalar_tensor_tensor(
                    vt[:, 0:wd], rz[:, 1, 0:wd], -1.0, nt[:, 0:wd],
                    op0=ALU.mult, op1=ALU.mult)
                ut = sbp.tile([128, 160], BF16, name="ut" + st,
                              tag="ut" + st)
                if k > 0:
                    nc.vector.tensor_add(ut[:, 0:wd], nt[:, 0:wd],
                                         zh[:, 0:wd])
                else:
                    nc.vector.tensor_copy(ut[:, 0:wd], nt[:, 0:wd])
                _uv[st] = (vt, ut)
                # h' = u + v (off-chain: payload emit / next round's zh)
                if e < 0:
                    hdst = sbp.tile([128, 160], BF16, name="hh" + st,
                                    tag="hh" + st)
                    _hh[st] = hdst
                    hdst = hdst[:, 0:wd]
                else:
                    hdst = enc[:, row, 0:wd // BC, :].rearrange(
                        "p c b -> p (c b)")
                    _hh[st] = hdst
                nc.gpsimd.tensor_add(hdst, ut[:, 0:wd], vt[:, 0:wd])

            _uv = {}
            _uv = {}
            _hh = {}
            # q-side attention prep, dribbled into the scan tail
            _qprep = []
            if _PH >= 3:
                def _q_repack(b, half, enc, r0, eng):
                    def f():
                        srcq = enc[:, r0:r0 + SQ_, NCP:NCP + NCQ, b] \
                            .rearrange("p s c -> p c s")
                        dstq = qencFB[:, half * NTQ + b * Q:
                                      half * NTQ + (b + 1) * Q]
                        if eng is nc.scalar:
                            eng.activation(dstq, srcq, AF.Copy)
                        else:
                            eng.tensor_scalar_add(dstq, srcq, 0.0)
                    return f

                qeng = [nc.scalar, nc.vector, nc.gpsimd]
                for b in range(BC):
                    for half, enc, r0 in ((0, encA, 0),
                                          (1, encB, SQ_ % SP_)):
                        _qprep.append(_q_repack(
                            b, half, enc, r0, qeng[(b * 2 + half) % 3]))

                def _qenc3(half):
                    def f():
                        nc.scalar.activation(
                            qenc3[:, half * NTQ:(half + 1) * NTQ],
                            qencFB[:, half * NTQ:(half + 1) * NTQ],
                            AF.Copy, scale=outw_sb[:, half:half + 1])
                    return f
                _qprep.append(_qenc3(0))
                _qprep.append(_qenc3(1))

                def _qwm():
                    pq = ps.tile([128, 512], F32, name="pqw", tag="pj",
                                 bufs=3)
                    nc.tensor.matmul(pq[0:1, 0:NTQ], sew_sb[:, 12:13],
                                     qencFB[:, 0:NTQ], start=True, stop=False)
                    nc.tensor.matmul(pq[0:1, 0:NTQ], sew_sb[:, 13:14],
                                     qencFB[:, NTQ:2 * NTQ], start=False,
                                     stop=True)
                    nc.vector.scalar_tensor_tensor(
                        qwm[0:1, :], qm0_sb[0:1, :], NEG, pq[0:1, 0:NTQ],
                        op0=ALU.mult, op1=ALU.add)
                _qprep.append(_qwm)

                def _qtr(b, half, eng):
                    def f():
                        ptr = ps.tile([128, 512], BF16, name="ptq", tag="tq",
                                      bufs=1)
                        nc.tensor.transpose(
                            ptr[0:Q, 0:HH],
                            qencFB[:, half * NTQ + b * Q:
                                   half * NTQ + (b + 1) * Q],
                            ident[:, :])
                        col = (b * 2 + half) * HH
                        if eng is nc.scalar:
                            eng.activation(qencT[0:Q, col:col + HH],
                                           ptr[0:Q, 0:HH], AF.Copy)
                        else:
                            eng.tensor_scalar_add(qencT[0:Q, col:col + HH],
                                                  ptr[0:Q, 0:HH], 0.0)
                    return f
                for b in range(BC):
                    for half in range(2):
                        _qprep.append(_qtr(b, half,
                                           qeng[(b * 2 + half) % 2]))

            NRND = RND if _PH >= 2 else 0
            for k in range(NRND):
                if KPRE_N == 1 and k == 1:
                    nc.tensor.wait_ge(aux_sem, 112)  # whh loaded
                first_half("A", k)
                if k > 0:
                    second_half("B", k - 1)
                first_half("B", k)
                second_half("A", k)
                if _pjobs:
                    _pjobs.pop(0)()
                if k == 0 and _pjobs:
                    _pjobs.pop(0)()
                if KPRE_N == 1 and k == W + SQ_:
                    for eng in (nc.scalar, nc.vector, nc.gpsimd, nc.tensor):
                        eng.wait_ge(aux_sem, 144)
                if k > W + SQ_:
                    for _ in range(4):
                        if _qprep:
                            _qprep.pop(0)()
            if NRND:
                second_half("B", NRND - 1)
            while _qprep:
                _qprep.pop(0)()

            if _PH < 3:
                nc.gpsimd.dma_start(out[0:16, :], encA[0:16, 0:4, 0:32, :])

            if _PH >= 3:
                # ---- attention ---- (q-side prep ran in the scan tail)
                # logits read enc directly (strided); no repack barrier
                def pv(enc, b):
                    return enc[:, :, 0:NCP, b].rearrange("p s c -> p c s")

                exT = {}
                for b in range(BC):
                    pt_ = ps.tile([128, 512], F32, name="plgT", tag="A")
                    o = pt_[0:Q, :]
                    nc.tensor.matmul(o, qenc3[:, b * Q:(b + 1) * Q],
                                     pv(encA, b), start=True, stop=False)
                    nc.tensor.matmul(o, qenc3[:, NTQ + b * Q:NTQ + (b + 1) * Q],
                                     pv(encB, b), start=False, stop=False)
                    nc.tensor.matmul(o, qwm[0:1, b * Q:(b + 1) * Q],
                                     ones_sb[0:1, 0:P], start=False, stop=True)
                    ex = sbp.tile([64, 512], BF16, name="exT", tag=f"exT{b}",
                                  bufs=1)
                    nc.scalar.activation(ex[0:Q, :], pt_[0:Q, :], AF.Exp)
                    exT[b] = ex

                # seA head group: penc terms + bias row (strided enc reads)
                seA = ps.tile([128, 512], F32, name="seA", tag="B")
                na = 0
                for b in range(BC):
                    for half, enc in ((0, encA), (1, encB)):
                        blk = (b * 6 + half) * 8
                        nc.tensor.matmul(seA[0:2 * BC, :],
                                         sew24_sb[:, blk:blk + 8],
                                         pv(enc, b), start=(na == 0),
                                         stop=False)
                        na += 1
                nc.tensor.matmul(seA[0:2 * BC, :], bhnr_sb[0:1, 512:520],
                                 ones_sb[0:1, 0:P], start=False, stop=True)

                # repack enc -> (b, t) for the elementwise paw path
                mvi = 0
                for b in range(BC):
                    for half, enc in ((0, encA), (1, encB)):
                        src = enc[:, :, 0:NCP, b].rearrange("p s c -> p c s")
                        dst = pencFB[:, half * NTP + b * P:half * NTP + (b + 1) * P]
                        eng = mveng[mvi % 3]
                        mvi += 1
                        if eng is nc.scalar:
                            eng.activation(dst, src, AF.Copy)
                        else:
                            eng.tensor_scalar_add(dst, src, 0.0)

                # per-(head,b) column sums into psum rows 0:8 (block-sparse ones)
                sm = ps.tile([128, 512], F32, name="sums", tag="pj", bufs=3)
                for b in range(BC):
                    nc.tensor.matmul(sm[0:2 * BC, :],
                                     ones8_sb[0:64, 8 * b:8 * b + 8],
                                     exT[b][0:Q, :],
                                     start=(b == 0), stop=(b == BC - 1))
                nc.vector.reciprocal(rs8_sb[:, :], sm[0:2 * BC, :])

                # attw_un[h, t] = qencT.T @ exT ; -> sbuf ; paw = penc*attw_un
                mvi = 0
                for b in range(BC):
                    for half in range(2):
                        pw = ps.tile([128, 512], F32, name="paw", tag="A")
                        col = (b * 2 + half) * HH
                        nc.tensor.matmul(pw[:, :],
                                         qencT[0:Q, col:col + HH],
                                         exT[b][0:Q, :],
                                         start=True, stop=True)
                        dst = attwFB[:, half * NTP + b * P:half * NTP + (b + 1) * P]
                        eng = mveng[mvi % 2]
                        mvi += 1
                        if eng is nc.scalar:
                            eng.activation(dst, pw[:, :], AF.Copy)
                        else:
                            eng.tensor_scalar_add(dst, pw[:, :], 0.0)
                for j in (0, 2, 1, 3):
                    sl = slice(j * NTP // 2, (j + 1) * NTP // 2)
                    eng = nc.vector if j != 1 else nc.gpsimd
                    eng.tensor_mul(pawFB[:, sl], pencFB[:, sl], attwFB[:, sl])

                # heads into psum rows 0:8 via block-sparse lhs weights
                # (sew24 block (b*6+j) is zero except cols {b, 4+b});
                # seA = penc terms + bias row; seB = attw/paw terms (scaled by rs8)
                seB = ps.tile([128, 512], F32, name="seB", tag="B")
                nb_ = 0
                for b in range(BC):
                    rhss = (attwFB[:, b * P:(b + 1) * P],
                            attwFB[:, NTP + b * P:NTP + (b + 1) * P],
                            pawFB[:, b * P:(b + 1) * P],
                            pawFB[:, NTP + b * P:NTP + (b + 1) * P])
                    for jj, rhs in enumerate(rhss):
                        blk = (b * 6 + jj + 2) * 8
                        nc.tensor.matmul(seB[0:2 * BC, :],
                                         sew24_sb[:, blk:blk + 8], rhs,
                                         start=(nb_ == 0),
                                         stop=(b == BC - 1 and jj == 3))
                        nb_ += 1
                t8 = sbp.tile([2 * BC, P], F32, name="t8", tag="t8")
                nc.vector.tensor_mul(t8[:, :], seB[0:2 * BC, :], rs8_sb[:, :])
                nc.vector.tensor_add(se8[:, :], seA[0:2 * BC, :], t8[:, :])
                nc.vector.copy_predicated(se8[:, :], m8_sb[:, :], neg_sb[:, :])
                nc.sync.dma_start(out[0:2 * BC, :], se8[:, :])
                nc.vector.tensor_reduce(red_sb[:, 1:2], se8[:, :], AX, ALU.max,
                                        negate=True)
                if V_ACCUM:
                    nc.scalar.activation(lse_sb[:, :], se8[:, :], AF.Exp,
                                         bias=red_sb[:, 1:2],
                                         accum_out=red_sb[:, 2:3])
                else:
                    nc.scalar.activation(lse_sb[:, :], se8[:, :], AF.Exp,
                                         bias=red_sb[:, 1:2])
                    nc.vector.tensor_reduce(red_sb[:, 2:3], lse_sb[:, :], AX,
                                            ALU.add)
                nc.scalar.activation(red_sb[:, 3:4], red_sb[:, 2:3], AF.Ln)
                nc.vector.tensor_sub(red_sb[:, 4:5], red_sb[:, 3:4],
                                     red_sb[:, 1:2])
                nc.vector.tensor_scalar(out=lsm_sb[:, :], in0=se8[:, :],
                                        scalar1=red_sb[:, 4:5], scalar2=None,
                                        op0=ALU.subtract)
                nc.sync.dma_start(out[2 * BC:4 * BC, :], lsm_sb[:, :])

    _split_multiwaits(nc)
    return nc, es


def _split_multiwaits(nc):
    """HW instruction encodings hold a single semaphore wait; move extra
    waits emitted by Tile onto same-engine NOPs inserted just before."""
    for b in nc.main_func.blocks:
        il = b.instructions
        newlist = []
        for inst in il:
            if type(inst).__name__ == "InstISA":
                # EVENT_SEMAPHORE_RANGE_CLEAR mis-encodes for this walrus
                # build; NRT clears semaphores per execution anyway.
                continue
            si = inst.sync_info
            if si is not None and len(si.on_wait) > 1:
                waits = list(si.on_wait)
                for wx in waits[:-1]:
                    nop = nc.engines[inst.engine].nop(hint="wsplit").ins
                    for bb in nc.main_func.blocks:
                        try:
                            bb.instructions.remove(nop)
                            break
                        except ValueError:
                            pass
                    nop.sync_info = mybir.SyncInfo(on_wait=[wx], on_update=[])
                    newlist.append(nop)
                inst.sync_info = mybir.SyncInfo(on_wait=[waits[-1]],
                                                on_update=list(si.on_update))
            newlist.append(inst)
        il[:] = newlist


def _prep_core(inputs, c):
    bs = slice(c * BC, (c + 1) * BC)
    ptok = np.asarray(inputs["passage"][bs]).astype(np.int64).reshape(-1)
    qtok = np.asarray(inputs["question"][bs]).astype(np.int64).reshape(-1)
    d = {}
    embp = inputs["_embp"]  # [VOCAB, 256 + E2R]
    ep = embp[ptok].T       # [256 + E2R, NTP]
    d["epTp_d"] = np.ascontiguousarray(
        ep[0:256].reshape(2, 128, NTP).transpose(1, 0, 2).reshape(128, -1))
    ep2 = np.zeros((E2, NTP), ep.dtype)
    ep2[0:E2R] = ep[256:256 + E2R]
    d["epTp2_d"] = ep2
    eq = embp[qtok].T
    d["epTq_d"] = np.ascontiguousarray(
        eq[0:256].reshape(2, 128, NTQ).transpose(1, 0, 2).reshape(128, -1))
    eq2 = np.zeros((E2, NTQ), eq.dtype)
    eq2[0:E2R] = eq[256:256 + E2R]
    d["epTq2_d"] = eq2
    qm0 = (qtok == 0).astype(np.float32)
    d["qm0"] = np.ascontiguousarray(qm0[None, :])
    pm2 = (ptok == 0).reshape(BC, P).astype(np.uint8)
    d["m8"] = np.ascontiguousarray(np.concatenate([pm2, pm2], axis=0))
    return d


def _prep_shared(inputs):
    bf = ml_dtypes.bfloat16

    wihT = np.zeros((4, 2, 128, 3 * HH), bf)      # (d, kc01, p, m)
    wih2T = np.zeros((4, E2, 3 * HH), bf)         # (d, p2, m)
    whhT = np.zeros((4, HH, 3 * HH), bf)          # (d, p, m)
    brzn = np.zeros((4, HH, 3), np.float32)
    bhnr = np.zeros((1, 576), bf)
    for di, (pre, dd) in enumerate((("p", "f"), ("p", "b"),
                                    ("q", "f"), ("q", "b"))):
        wih = np.asarray(inputs[f"{pre}_wih_{dd}"], np.float32)
        whh = np.asarray(inputs[f"{pre}_whh_{dd}"], np.float32)
        bih = np.asarray(inputs[f"{pre}_bih_{dd}"], np.float32)
        bhh = np.asarray(inputs[f"{pre}_bhh_{dd}"], np.float32)
        wT = np.zeros((EPAD, 3 * HH), bf)
        wT[:E, :] = wih.T.astype(bf)
        wT[E, HH:2 * HH] = BIGM  # pad-token mask column -> z-gate freeze
        wihT[di] = wT[0:256].reshape(2, 128, 3 * HH)
        wih2T[di, 0:E2R] = wT[256:256 + E2R]
        whhT[di] = whh.T.astype(bf)
        for gg in range(3):
            brzn[di, :, gg] = bih[gg * HH:(gg + 1) * HH] + (
                bhh[gg * HH:(gg + 1) * HH] if gg < 2 else 0)
        bhnr[0, di * HH:(di + 1) * HH] = bhh[2 * HH:].astype(bf)
    wihT = np.ascontiguousarray(
        wihT.transpose(2, 0, 1, 3).reshape(128, -1))      # (p,(d,kc,m))
    wih2T = np.ascontiguousarray(
        wih2T.transpose(1, 0, 2).reshape(E2, -1))         # (p2,(d,m))
    whhT = np.ascontiguousarray(
        whhT.transpose(1, 0, 2).reshape(128, -1))         # (p,(d,m))
    brzn = np.ascontiguousarray(brzn.transpose(1, 0, 2).reshape(128, 12))

    aw = np.asarray(inputs["attn_w"], np.float32)
    w2, w3 = aw[256:512], aw[512:]
    outw = np.zeros((HH, 4), np.float32)
    outw[:, 0], outw[:, 1] = w3[:128], w3[128:]
    outw[0:BC, 2] = float(np.asarray(inputs["start_b"]))
    outw[BC:2 * BC, 2] = float(np.asarray(inputs["end_b"]))

    sw = np.asarray(inputs["start_w"], np.float32)
    ew = np.asarray(inputs["end_w"], np.float32)
    sew = np.zeros((HH, 14), bf)
    for j in range(6):
        sew[:, 2 * j] = sw[j * 128:(j + 1) * 128].astype(bf)
        sew[:, 2 * j + 1] = ew[j * 128:(j + 1) * 128].astype(bf)
    sew[:, 12] = w2[:128].astype(bf)
    sew[:, 13] = w2[128:].astype(bf)
    sew24 = np.zeros((HH, 192), bf)
    for b in range(BC):
        for j in range(6):
            blk = (b * 6 + j) * 8
            sew24[:, blk + b] = sw[j * 128:(j + 1) * 128].astype(bf)
            sew24[:, blk + BC + b] = ew[j * 128:(j + 1) * 128].astype(bf)
    ones8 = np.zeros((128, 32), bf)
    for b in range(BC):
        ones8[:, 8 * b + b] = 1.0
        ones8[:, 8 * b + BC + b] = 1.0
    bhnr[0, 512:512 + BC] = np.float32(inputs["start_b"]).astype(bf)
    bhnr[0, 516:516 + BC] = np.float32(inputs["end_b"]).astype(bf)
    return {"wihT": wihT, "wih2T": wih2T, "whhT": whhT, "brzn": brzn,
            "bhnr": bhnr,
            "outw": outw, "sew": sew, "sew24": sew24, "ones8": ones8}


def kernel(**inputs):
    if "nc" not in _CACHE:
        _CACHE["nc"] = _build_nc()
    nc, _es = _CACHE["nc"]
    shared = _prep_shared(inputs)
    bf = ml_dtypes.bfloat16
    embp = np.zeros((VOCAB, 256 + E2R), bf)
    embp[:, :E] = np.asarray(inputs["emb"], np.float32).astype(bf)
    embp[0, E] = 1.0  # pad-token indicator column
    inputs = dict(inputs)
    inputs["_embp"] = embp
    in_maps = []
    for c in range(NC):
        m = dict(shared)
        m.update(_prep_core(inputs, c))
        in_maps.append(m)
    res = run_bass_kernel_spmd(nc, in_maps, list(range(NC)))
    outs = [np.asarray(res.results[c]["out"]) for c in range(NC)]
    se = np.concatenate([o[0:2 * BC].reshape(2, BC, P) for o in outs], axis=1)
    lsm = np.concatenate([o[2 * BC:].reshape(2, BC, P) for o in outs], axis=1)
    return (np.ascontiguousarray(se[0]), np.ascontiguousarray(se[1]),
            np.ascontiguousarray(lsm[0]), np.ascontiguousarray(lsm[1]))

